# revision 15
# baseline (speedup 1.0000x reference)
"""DistMult edge scoring on 8 Trainium2 NeuronCores.

score[e] = sum_d node_emb[src[e], d] * rel_emb[e, d] * node_emb[dst[e], d]

Strategy (v2): exact 256B-row gathers on 4 SWDGE queues.
  - Edges are sorted globally by src and sharded contiguously, so each
    core's src ids span a ~12.5k-row window of the table. The host ships
    that window as a per-core "slice" input; src gather indices are
    slice-local (< 16384, fits the gather's int16 index format) and fetch
    exact 256B rows - no group amplification, no mask-select.
  - dst ids are random over all 100k nodes. int16 indices address at most
    32768 rows, and descriptors must be 256B-multiples, so the table is
    shipped as [50000, 128] row-pairs and each edge's dst is gathered as
    an exact 64-float row from one of 4 static views (half x parity):
    table2[0|25000:+25000, 0:64|64:128]. Within a tile, edges are dealt
    into 4 fixed-capacity class segments (CAP each) so each class is one
    contiguous gather call; pad slots gather row 0 and are dropped on the
    host (their rel is zeroed).
  - dma_gather descgen runs at ~7.9 ns/idx on the Q7 cpu pair selected by
    queue_num. Queue 0 occupies the GpSimd engine for the whole descgen;
    queues 1-3 return in ~0.5us and generate in the background. Tiles are
    issued in pairs with calls spread over all 4 queues/pairs; each
    (parity, dir) semaphore stream stays on one queue so completions are
    FIFO per semaphore.
  - DVE per tile: u = head*rel; u *= tail; reduce -> scores. 3 ops.
"""

from contextlib import ExitStack

import numpy as np

import concourse.bacc as bacc
import concourse.bass as bass
import concourse.mybir as mybir
from concourse import library_config
from concourse.bass_utils import run_bass_kernel_spmd

N_NODES = 100000
DIM = 64
N_EDGES = 1000000
N_CORES = 8

P = 128
EPC = N_EDGES // N_CORES          # 125000
CAP = 1792                        # slots per dst class per tile (14*128); 113 descs/ring stays under the 128-deep SWDGE ring
KC = CAP // P                     # 9 k-slots per class
NCLS = 4                          # dst classes: (half, parity)
TILE = NCLS * CAP                 # 4608 slots per tile
KP = TILE // P                    # 36
NT = 18                           # tiles per core (18*1792 >= 125000/4 + slack)
SIF = CAP // 16                   # 72 int16 per partition per src call
DIF = CAP // 16                   # 72 per dst call
SLICE_ROWS = 16384
HALF = 50000                      # dst half split (even, so parity survives)

F32 = mybir.dt.float32
I16 = mybir.dt.int16

_cache = {}


def _build_program():
    if "nc" in _cache:
        return _cache["nc"]

    nc = bacc.Bacc(
        "TRN2",
        target_bir_lowering=False,
        debug=False,
        enable_asserts=False,
        num_devices=N_CORES,
        num_swdge_queues=4,
    )
    slice_h = nc.dram_tensor("slice", [SLICE_ROWS, DIM], F32, kind="ExternalInput")
    table2 = nc.dram_tensor("table2", [2 * HALF // 2, 2 * DIM], F32,
                            kind="ExternalInput")  # [50000, 128] row pairs
    sidx_h = nc.dram_tensor("sidx", [NT, P, NCLS, SIF], I16, kind="ExternalInput")
    didx_h = nc.dram_tensor("didx", [NT, P, NCLS, DIF], I16, kind="ExternalInput")
    rel_h = nc.dram_tensor("rel", [NT, P, KP, DIM], F32, kind="ExternalInput")
    out_h = nc.dram_tensor("score", [NT, P, KP], F32, kind="ExternalOutput")

    # dst gather sources: (half, parity) -> rows [h*25000:+25000], cols
    # [p*64:+64] of table2; row stride 128 elems = elem_step.
    dviews = [
        table2[h * (HALF // 2):(h + 1) * (HALF // 2), p * DIM:(p + 1) * DIM]
        for h in range(2) for p in range(2)
    ]

    NB = 4      # gather/compute tile slots
    NB_IO = 4   # idx/rel prefetch slots
    NBS = 4     # score output slots
    NOPS = 3    # DVE ops per tile

    with ExitStack() as stack:
        block = stack.enter_context(nc.Block())
        srcb = stack.enter_context(nc.sbuf_tensor("srcb", [P, NB, KP, DIM], F32))
        dstb = stack.enter_context(nc.sbuf_tensor("dstb", [P, NB, KP, DIM], F32))
        relb = stack.enter_context(nc.sbuf_tensor("relb", [P, NB_IO, KP, DIM], F32))
        sidxb = stack.enter_context(
            nc.sbuf_tensor("sidxb", [P, NB_IO, NCLS, SIF], I16)
        )
        didxb = stack.enter_context(
            nc.sbuf_tensor("didxb", [P, NB_IO, NCLS, DIF], I16)
        )
        scob = stack.enter_context(nc.sbuf_tensor("scob", [P, NBS, KP], F32))
        sem = lambda n: stack.enter_context(nc.semaphore(n))
        s_sidx, s_didx, s_rel = sem("s_sidx"), sem("s_didx"), sem("s_rel")
        # one gather sem per in-flight slot per direction: slot-reuse
        # gating (s_vc) orders increments within each sem. even tiles on
        # q0/q1, odd on q2/q3.
        s_s = tuple(sem(f"s_s{i}") for i in range(NB))
        s_d = tuple(sem(f"s_d{i}") for i in range(NB))
        s_vc, s_out = sem("s_vc"), sem("s_out")

        @block.sync
        def _(sp: bass.BassEngine):
            # pure prefetcher; completion counts stay ordered by tile
            for t in range(NT):
                s = t % NB_IO
                if t >= 1:
                    sp.wait_ge(s_sidx, 16 * t)
                    sp.wait_ge(s_didx, 16 * t)
                    sp.wait_ge(s_rel, 16 * t)
                if t >= NB_IO:
                    # idx slots free once tile t-NB_IO's gathers retired
                    tt = t - NB_IO
                    sp.wait_ge(s_s[tt % NB], 64 * (tt // NB + 1))
                    sp.wait_ge(s_d[tt % NB], 64 * (tt // NB + 1))
                    # rel slot consumed by DVE of tile t-NB_IO
                    sp.wait_ge(s_vc, NOPS * (tt + 1))
                sp.dma_start(out=sidxb[:, s], in_=sidx_h[t]).then_inc(s_sidx, 16)
                sp.dma_start(out=didxb[:, s], in_=didx_h[t]).then_inc(s_didx, 16)
                sp.dma_start(out=relb[:, s], in_=rel_h[t]).then_inc(s_rel, 16)

        @block.scalar
        def _(sc: bass.BassEngine):
            for t in range(NT):
                sc.wait_ge(s_vc, NOPS * (t + 1))
                if t >= 1:
                    sc.wait_ge(s_out, 16 * t)
                sc.dma_start(out=out_h[t], in_=scob[:, t % NBS]).then_inc(
                    s_out, 16
                )
            sc.wait_ge(s_out, 16 * NT)

        @block.gpsimd
        def _(gp: bass.BassGpSimd):
            gp.load_library(library_config.mlp)

            def dst_call(t, c):
                s = t % NB
                gp.dma_gather(
                    dstb[:, s, c * KC:(c + 1) * KC],
                    dviews[c],
                    didxb[:, t % NB_IO, c],
                    CAP,
                    CAP,
                    DIM,
                    elem_step=2 * DIM,
                    single_packet=False,
                    queue_num=1 if t % 2 == 0 else 2,
                ).then_inc(s_d[t % NB], 16)

            def src_call(t, c):
                s = t % NB
                gp.dma_gather(
                    srcb[:, s, c * KC:(c + 1) * KC],
                    slice_h[:],
                    sidxb[:, t % NB_IO, c],
                    CAP,
                    CAP,
                    DIM,
                    elem_step=DIM,
                    single_packet=False,
                    queue_num=0 if t % 2 == 0 else 3,
                ).then_inc(s_s[t % NB], 16)

            # issue in tile pairs, strict queue round-robin q1,q2,q3,q0.
            # A call to a busy Q7 pair stalls the engine until that pair
            # frees, so equal-size calls rotating over all 4 pairs keep
            # every pair fed; effective rate = 4 pairs in parallel.
            for t0 in range(0, NT, 2):
                t1 = t0 + 1
                gp.wait_ge(s_sidx, 16 * (t1 + 1))
                gp.wait_ge(s_didx, 16 * (t1 + 1))
                if t1 >= NB:
                    gp.wait_ge(s_vc, NOPS * (t1 - NB + 1))
                for c in range(NCLS):
                    dst_call(t0, c)   # q1
                    dst_call(t1, c)   # q2
                    src_call(t1, c)   # q3
                    src_call(t0, c)   # q0

        @block.vector
        def _(v: bass.BassEngine):
            mult = mybir.AluOpType.mult
            add = mybir.AluOpType.add
            for t in range(NT):
                s = t % NB
                v.wait_ge(s_s[t % NB], 64 * (t // NB + 1))
                v.wait_ge(s_d[t % NB], 64 * (t // NB + 1))
                v.wait_ge(s_rel, 16 * (t + 1))
                if t >= NBS:
                    v.wait_ge(s_out, 16 * (t - NBS + 1))
                if t >= 1:
                    v.wait_ge(s_vc, NOPS * t)
                i = NOPS * t
                v.tensor_tensor(
                    out=srcb[:, s], in0=srcb[:, s], in1=relb[:, t % NB_IO],
                    op=mult,
                ).then_inc(s_vc, 1)
                v.wait_ge(s_vc, i + 1)
                v.tensor_tensor(
                    out=srcb[:, s], in0=srcb[:, s], in1=dstb[:, s], op=mult
                ).then_inc(s_vc, 1)
                v.wait_ge(s_vc, i + 2)
                v.tensor_reduce(
                    out=scob[:, t % NBS],
                    in_=srcb[:, s],
                    axis=mybir.AxisListType.X,
                    op=add,
                ).then_inc(s_vc, 1)

    nc.compile()
    _cache["nc"] = nc
    return nc


def _wrap16(vals):
    """[..., n] int idx -> wrapped [..., 16, n // 16] replicated to 128
    partitions: idx j sits at [j % 16, j // 16]."""
    n = vals.shape[-1]
    lead = vals.shape[:-1]
    w = vals.reshape(*lead, n // 16, 16)
    w = np.swapaxes(w, -1, -2)  # [..., 16, n//16]
    w = np.broadcast_to(
        w[..., None, :, :], (*lead, 8, 16, n // 16)
    ).reshape(*lead, P, n // 16)
    return np.ascontiguousarray(w.astype(np.int16))


def _shard_inputs(node_emb, rel_emb, src, dst):
    node_emb = np.asarray(node_emb, dtype=np.float32)
    rel_emb = np.asarray(rel_emb, dtype=np.float32)
    src = np.asarray(src, dtype=np.int64)
    dst = np.asarray(dst, dtype=np.int64)

    table2 = np.ascontiguousarray(node_emb.reshape(HALF, 2 * DIM))
    order = np.argsort(src, kind="stable")

    in_maps = []
    slot2edge = []
    for c in range(N_CORES):
        eids = order[c * EPC:(c + 1) * EPC]
        s_c = src[eids]
        d_c = dst[eids]
        lo = int(s_c[0])
        span = int(s_c[-1]) - lo + 1
        assert span <= SLICE_ROWS, f"core {c} src span {span}"
        slc = np.zeros((SLICE_ROWS, DIM), np.float32)
        avail = min(SLICE_ROWS, N_NODES - lo)
        slc[:avail] = node_emb[lo:lo + avail]

        cls = (d_c >= HALF) * 2 + (d_c & 1)
        # class-local dst index
        dloc = np.where(d_c >= HALF, (d_c - HALF) >> 1, d_c >> 1)

        # deal each class into NT fixed-capacity tile segments
        slots = np.full((NT, NCLS, CAP), -1, np.int64)  # edge position in eids
        for k in range(NCLS):
            pos = np.nonzero(cls == k)[0]
            assert len(pos) <= NT * CAP, f"class {k} count {len(pos)}"
            flat = slots[:, k, :].reshape(-1)
            flat[:len(pos)] = pos
            slots[:, k, :] = flat.reshape(NT, CAP)

        valid = slots >= 0
        pos_safe = np.where(valid, slots, 0)

        sidx_v = np.where(valid, s_c[pos_safe] - lo, 0)   # [NT, NCLS, CAP]
        didx_v = np.where(valid, dloc[pos_safe], 0)
        rel_v = np.where(
            valid[..., None], rel_emb[eids[pos_safe]], 0.0
        ).astype(np.float32)                               # [NT, NCLS, CAP, D]

        sidx = _wrap16(sidx_v)                             # [NT, NCLS, P, SIF]
        didx = _wrap16(didx_v)                             # [NT, NCLS, P, DIF]

        # slot j of tile t -> (p=j%128, k=j//128); rel must sit at [p, k]
        rel_t = np.ascontiguousarray(
            rel_v.reshape(NT, KP, P, DIM).swapaxes(1, 2)
        )

        in_maps.append(
            {
                "slice": slc,
                "table2": table2,
                "sidx": np.ascontiguousarray(sidx.swapaxes(1, 2)),
                "didx": np.ascontiguousarray(didx.swapaxes(1, 2)),
                "rel": rel_t,
            }
        )
        slot2edge.append(np.where(valid, eids[pos_safe], -1))
    return in_maps, slot2edge


def _unshard(results, slot2edge):
    scores = np.empty(N_EDGES, np.float32)
    for c in range(N_CORES):
        flat = (
            np.asarray(results[c]["score"]).transpose(0, 2, 1).reshape(-1)
        )  # slot j = k*128+p order
        s2e = slot2edge[c].reshape(-1)
        m = s2e >= 0
        scores[s2e[m]] = flat[m]
    return scores


def run_on_hw(node_emb, rel_emb, src, dst, **spmd_kwargs):
    nc = _build_program()
    in_maps, slot2edge = _shard_inputs(node_emb, rel_emb, src, dst)
    res = run_bass_kernel_spmd(nc, in_maps, list(range(N_CORES)), **spmd_kwargs)
    return _unshard(res.results, slot2edge), res


def kernel(node_emb, rel_emb, src, dst):
    scores, _ = run_on_hw(node_emb, rel_emb, src, dst)
    return scores
